# revision 7
# baseline (speedup 1.0000x reference)
"""Domain-specific batchnorm (DSBatchNorm2 2D path) on 8 Trainium2 cores.

Strategy: feature-parallel sharding, fp16 input, int8 output. Core c owns
features [c*128,(c+1)*128) and sees ALL cells for them, so per-domain stats
need no cross-core reduction. The host sorts cells by domain and ships each
core a transposed shard [128 features, npad cells] of fp16. The normalized
output returns as int8 with a global scale s_out (1 byte instead of 2 on
the store side; rel err budget 2e-2 >> int8 quant err ~5e-3).

Engine assignment per domain block (measured TRN2 rates, ns/col/128 lanes):
  sum(x^2): ScalarE Square+accum, one instr/block        (0.87)
  sum(x):   DVE tensor_add halving cascade (fp16 2x) x3
            + tensor_scalar+accum tail                    (~0.61)
  pass2 out=x*a+b -> int8: DVE tensor_scalar 2x (0.55) with slices moved
            to GPSIMD tensor_scalar (G) and ScalarE Identity (Y) to balance
  finalize: 6 small DVE ops + 1 Sqrt
count==1 domains -> out = x: a = 1/s_out, b = 0. count==0 -> 0.
"""

import os

import numpy as np

import concourse.bass as bass
import concourse.tile as tile
from concourse import bacc, mybir
from concourse.bass_utils import run_bass_kernel_spmd

N_DOMAIN = 8
EPS = 1e-5
NCORES = 8
P = 128  # SBUF partitions = features per core
ALIGN = 64  # domain block alignment (columns)
OUT_MARGIN = 1.25  # headroom on the output quant range

# pass2 splits (fractions of each domain's columns)
G_GPS = float(os.environ.get("DSBN_G", "0.25"))  # pass2 share on GPSIMD
Y_ACT = float(os.environ.get("DSBN_Y", "0.075"))  # pass2 share on ScalarE
G_LAST = float(os.environ.get("DSBN_GL", "0.34"))  # last-domain pass2 GPS share
Y_LAST = float(os.environ.get("DSBN_YL", "0.18"))  # last-domain pass2 ACT share
# domains whose sum(x) runs on ScalarE Copy+accum instead of the DVE cascade
SX_ACT = {int(t) for t in os.environ.get("DSBN_SXACT", "").split(",") if t != ""}

_cache: dict = {}


class _Plan:
    pass


def _plan(y: np.ndarray) -> _Plan:
    p = _Plan()
    y = np.asarray(y).astype(np.int64).ravel()
    n = y.shape[0]
    p.n = n
    p.counts = np.bincount(y, minlength=N_DOMAIN).astype(np.int64)
    p.order = np.argsort(y, kind="stable")
    blk = np.maximum((p.counts + ALIGN - 1) // ALIGN * ALIGN, ALIGN)
    p.blk = blk
    p.npad = int(blk.sum())
    bstart = np.concatenate([[0], np.cumsum(blk)])[:-1]
    p.bstart = bstart
    cstart = np.concatenate([[0], np.cumsum(p.counts)])[:-1]
    col_idx = np.empty(n, dtype=np.int64)
    for d in range(N_DOMAIN):
        col_idx[cstart[d] : cstart[d] + p.counts[d]] = bstart[d] + np.arange(
            p.counts[d]
        )
    p.col_idx = col_idx
    return p


def _split(L, frac):
    k = int(round(frac * L / ALIGN)) * ALIGN
    return max(0, min(k, L))


def _build(plan: _Plan):
    f32 = mybir.dt.float32
    f16 = mybir.dt.float16
    i8 = mybir.dt.int8
    A = mybir.AluOpType
    AF = mybir.ActivationFunctionType
    D = N_DOMAIN
    npad = plan.npad
    blk = [int(b) for b in plan.blk]
    bstart = [int(b) for b in plan.bstart]
    lmax = max(blk)

    nc = bacc.Bacc("TRN2", target_bir_lowering=False, debug=False, num_devices=NCORES)
    xt = nc.dram_tensor("xt", [P, npad], f16, kind="ExternalInput").ap()
    cmat = nc.dram_tensor("cmat", [P, 8], f32, kind="ExternalInput").ap()
    outd = nc.dram_tensor("out", [P, npad], i8, kind="ExternalOutput").ap()

    with tile.TileContext(nc) as tc:
        with tc.tile_pool(name="const", bufs=1) as const_p, \
             tc.tile_pool(name="qres", bufs=1) as qres_p, \
             tc.tile_pool(name="hscr", bufs=2) as h_p, \
             tc.tile_pool(name="junk", bufs=1) as junk_p, \
             tc.tile_pool(name="st", bufs=1) as st_p, \
             tc.tile_pool(name="fin", bufs=1) as fin_p, \
             tc.tile_pool(name="ot", bufs=2) as out_p:

            cm = const_p.tile([P, 8], f32, tag="cm", name="cm")
            nc.sync.dma_start(cm[:], cmat)
            gammap = cm[:, 0:1]   # gamma_f / s_out
            betap = cm[:, 1:2]    # beta_f / s_out
            epsp = cm[:, 2:3]     # EPS
            sratio = cm[:, 3:4]   # 1 / s_out

            # dummy Sqrt up front: pulls the ACT table load into the DMA ramp
            warm = const_p.tile([P, 1], f32, tag="warm", name="warm")
            nc.scalar.activation(warm[:], epsp, AF.Sqrt, bias=epsp, scale=1.0)

            xd = []
            for d in range(D):
                t = qres_p.tile([P, blk[d]], f16, tag=f"x{d}", name=f"x{d}")
                nc.sync.dma_start(t[:], xt[:, bstart[d] : bstart[d] + blk[d]])
                xd.append(t)

            p1 = [st_p.tile([P, 2], f32, tag=f"p1_{d}", name=f"p1_{d}") for d in range(D)]
            p2 = [st_p.tile([P, 2], f32, tag=f"p2_{d}", name=f"p2_{d}") for d in range(D)]
            av = [fin_p.tile([P, 1], f32, tag=f"av_{d}", name=f"av_{d}") for d in range(D)]
            bv = [fin_p.tile([P, 1], f32, tag=f"bv_{d}", name=f"bv_{d}") for d in range(D)]

            def sx_cascade(d, s, e, slot):
                # sum(x[:, s:e]) via 3-level fp16 halving + short reduce
                L = e - s
                L2, L4, L8 = L // 2, L // 4, L // 8
                h = h_p.tile([P, L2 + L4 + L8], f16, tag="h", name=f"h_{d}_{slot}")
                nc.vector.tensor_add(h[:, :L2], xd[d][:, s : s + L2], xd[d][:, s + L2 : e])
                nc.vector.tensor_add(h[:, L2 : L2 + L4], h[:, :L4], h[:, L4:L2])
                nc.vector.tensor_add(
                    h[:, L2 + L4 :], h[:, L2 : L2 + L8], h[:, L2 + L8 : L2 + L4]
                )
                nc.vector.tensor_scalar(
                    out=h[:, L2 + L4 :], in0=h[:, L2 + L4 :], scalar1=1.0,
                    scalar2=None, op0=A.mult, op1=A.add,
                    accum_out=p1[d][:, slot : slot + 1],
                )

            def stats(d, split_sq=False, halves=False):
                L = blk[d]
                ranges = [(0, _split(L, 0.5)), (_split(L, 0.5), L)] if halves else [(0, L)]
                sqj = junk_p.tile([P, lmax], i8, tag="sqj", name=f"sqj_{d}")
                if split_sq:
                    hsp = _split(L, 0.5)
                    nc.scalar.activation(
                        sqj[:, :hsp], xd[d][:, :hsp], AF.Square,
                        accum_out=p2[d][:, 0:1],
                    )
                    sqf = junk_p.tile([P, lmax // 2], f16, tag="sqf", name=f"sqf_{d}")
                    nc.vector.scalar_tensor_tensor(
                        out=sqf[:, : L - hsp], in0=xd[d][:, hsp:], scalar=1.0,
                        in1=xd[d][:, hsp:], op0=A.mult, op1=A.mult,
                        accum_out=p2[d][:, 1:2],
                    )
                else:
                    for i, (s, e) in enumerate(ranges):
                        nc.scalar.activation(
                            sqj[:, s:e], xd[d][:, s:e], AF.Square,
                            accum_out=p2[d][:, i : i + 1],
                        )
                    if len(ranges) == 1:
                        nc.vector.memset(p2[d][:, 1:2], 0.0)
                if d in SX_ACT:
                    cpj = junk_p.tile([P, lmax], i8, tag="cpj", name=f"cpj_{d}")
                    nc.scalar.activation(
                        cpj[:, :L], xd[d][:], AF.Copy, accum_out=p1[d][:, 0:1]
                    )
                    nc.vector.memset(p1[d][:, 1:2], 0.0)
                else:
                    for i, (s, e) in enumerate(ranges):
                        sx_cascade(d, s, e, i)
                    if len(ranges) == 1:
                        nc.vector.memset(p1[d][:, 1:2], 0.0)

            def finalize(d):
                c = float(plan.counts[d])
                if c < 1.5:
                    if c < 0.5:
                        nc.vector.memset(av[d][:], 0.0)
                    else:
                        nc.vector.tensor_copy(av[d][:], sratio)
                    nc.vector.memset(bv[d][:], 0.0)
                    return
                s1 = fin_p.tile([P, 1], f32, tag=f"s1_{d}", name=f"s1_{d}")
                nc.vector.tensor_add(s1[:], p1[d][:, 0:1], p1[d][:, 1:2])
                mneg = fin_p.tile([P, 1], f32, tag=f"mn_{d}", name=f"mn_{d}")
                nc.vector.tensor_scalar(
                    out=mneg[:], in0=s1[:], scalar1=-1.0 / c, scalar2=None,
                    op0=A.mult,
                )
                s2 = fin_p.tile([P, 1], f32, tag=f"s2_{d}", name=f"s2_{d}")
                nc.vector.tensor_add(s2[:], p2[d][:, 0:1], p2[d][:, 1:2])
                m2 = fin_p.tile([P, 1], f32, tag=f"m2_{d}", name=f"m2_{d}")
                nc.vector.tensor_mul(m2[:], mneg[:], mneg[:])
                var = fin_p.tile([P, 1], f32, tag=f"va_{d}", name=f"va_{d}")
                nc.vector.scalar_tensor_tensor(
                    out=var[:], in0=s2[:], scalar=1.0 / c, in1=m2[:],
                    op0=A.mult, op1=A.subtract,
                )
                std = fin_p.tile([P, 1], f32, tag=f"sd_{d}", name=f"sd_{d}")
                nc.scalar.activation(std[:], var[:], AF.Sqrt, bias=epsp, scale=1.0)
                rstd = fin_p.tile([P, 1], f32, tag=f"rs_{d}", name=f"rs_{d}")
                nc.vector.reciprocal(rstd[:], std[:])
                nc.vector.tensor_mul(av[d][:], rstd[:], gammap)
                nc.vector.scalar_tensor_tensor(
                    out=bv[d][:], in0=mneg[:], scalar=av[d][:, 0:1], in1=betap,
                    op0=A.mult, op1=A.add,
                )

            def pass2(d, g_frac=None, y_frac=None):
                L = blk[d]
                g = _split(L, G_GPS if g_frac is None else g_frac)
                y = _split(L, Y_ACT if y_frac is None else y_frac)
                if g + y > L:
                    y = L - g
                ot = out_p.tile([P, lmax], i8, tag="ot", name=f"ot_{d}")
                lo = 0
                if g > 0:
                    nc.gpsimd.tensor_scalar(
                        out=ot[:, :g], in0=xd[d][:, :g],
                        scalar1=av[d][:, 0:1], scalar2=bv[d][:, 0:1],
                        op0=A.mult, op1=A.add,
                    )
                    lo = g
                if y > 0:
                    nc.scalar.activation(
                        ot[:, lo : lo + y], xd[d][:, lo : lo + y], AF.Identity,
                        bias=bv[d][:, 0:1], scale=av[d][:, 0:1],
                    )
                    lo += y
                if lo < L:
                    nc.vector.tensor_scalar(
                        out=ot[:, lo:L],
                        in0=xd[d][:, lo:L],
                        scalar1=av[d][:, 0:1],
                        scalar2=bv[d][:, 0:1],
                        op0=A.mult,
                        op1=A.add,
                    )
                nc.sync.dma_start(outd[:, bstart[d] : bstart[d] + L], ot[:, :L])

            for d in range(D):
                stats(d, split_sq=(d == D - 1), halves=(d == 0))
                if d >= 1:
                    finalize(d - 1)
                    pass2(d - 1)
            finalize(D - 1)
            # larger off-DVE share for the last block: shorter drain tail
            pass2(D - 1, g_frac=G_LAST, y_frac=Y_LAST)

    nc.compile()
    return nc


def _prepare(x, y, gamma, beta):
    x = np.asarray(x)
    if x.dtype != np.float32:
        x = x.astype(np.float32)
    yv = np.asarray(y)
    g = np.asarray(gamma, dtype=np.float32).reshape(-1)
    b = np.asarray(beta, dtype=np.float32).reshape(-1)
    n, f = x.shape
    assert f == P * NCORES, f"expected {P * NCORES} features, got {f}"

    key = (n, f, hash(yv.tobytes()))
    if key in _cache:
        nc, plan = _cache[key]
    else:
        plan = _plan(yv)
        nc = _build(plan)
        _cache.clear()
        _cache[key] = (nc, plan)

    absmax = float(np.abs(x).max())
    s_out = (
        OUT_MARGIN * absmax * max(float(np.abs(g).max()), 1e-30)
        + float(np.abs(b).max())
    ) / 127.0
    plan.s_out = s_out

    # padded, domain-sorted cell matrix [npad, f] fp16
    Xp = np.zeros((plan.npad, f), dtype=np.float16)
    Xp[plan.col_idx] = x[plan.order].astype(np.float16)

    in_maps = []
    for c in range(NCORES):
        sl = slice(c * P, (c + 1) * P)
        xc = np.ascontiguousarray(Xp[:, sl].T)  # [128, npad] fp16
        cmat = np.zeros((P, 8), dtype=np.float32)
        cmat[:, 0] = g[sl] / s_out
        cmat[:, 1] = b[sl] / s_out
        cmat[:, 2] = EPS
        cmat[:, 3] = 1.0 / s_out
        in_maps.append({"xt": xc, "cmat": cmat})
    return nc, plan, in_maps, n, f


def _finish(results, plan, n, f):
    out = np.empty((n, f), dtype=np.float32)
    for c in range(NCORES):
        oc = results[c]["out"]  # [128, npad] int8
        out[plan.order, c * P : (c + 1) * P] = (
            oc[:, plan.col_idx].T.astype(np.float32) * plan.s_out
        )
    return out


def _finish_core0(oc, plan, n):
    out0 = np.empty((n, P), dtype=np.float32)
    out0[plan.order] = oc[:, plan.col_idx].T.astype(np.float32) * plan.s_out
    return out0


def kernel(x, y, gamma, beta):
    nc, plan, in_maps, n, f = _prepare(x, y, gamma, beta)
    res = run_bass_kernel_spmd(nc, in_maps, list(range(NCORES)))
    return _finish(res.results, plan, n, f)


def run_profiled(x, y, gamma, beta):
    """Like kernel() but with NTFF tracing; returns (out, BassKernelResults)."""
    nc, plan, in_maps, n, f = _prepare(x, y, gamma, beta)
    res = run_bass_kernel_spmd(nc, in_maps, list(range(NCORES)), trace=True)
    return _finish(res.results, plan, n, f), res
